# revision 1
# baseline (speedup 1.0000x reference)
"""DeepseekV2 MoE layer on 8 Trainium2 NeuronCores.

Strategy (expert-parallel, per the sharding hint):
  - Router gate + grouped top-k computed on host (0.03% of module FLOPs);
    it determines the dispatch, which IS the input sharding.
  - 16 routed experts paired big-count-with-small-count onto 8 cores
    (2 experts per core, token lists gathered host-side, padded to a
    shared per-slot capacity so all cores run one SPMD program).
  - Shared-expert MLP is data-parallel over tokens: each core runs
    T/8 = 512 tokens through the full shared MLP.
  - All matmuls in bf16 (fp32 PE matmul is 2x slower), f32 PSUM
    accumulation, f32 outputs.
  - Device computes outputs token-on-free-dim (transposed); host
    transposes/combines during unshard.
"""

import sys

sys.path.insert(0, "/opt/trn_rl_repo")

import copy

import ml_dtypes
import numpy as np

import concourse.bass as bass
import concourse.mybir as mybir
import concourse.tile as tile
from concourse.bass_utils import run_bass_kernel_spmd

DT = mybir.dt
BF16 = ml_dtypes.bfloat16

T, D, E, I = 4096, 2048, 16, 1024
TOP_K, N_GROUP, TOPK_GROUP = 4, 4, 2
ROUTED_SCALE = 2.5
SHARED_I = 2048
N_CORES = 8
P = 128
NCHUNK = 512  # token chunk (matmul moving free dim)


# ---------------------------------------------------------------- wait split
def _split_excess_waits(nc, limit=1):
    """This walrus build rejects >1 sync-wait command per instruction.
    Move excess waits onto fresh same-engine NOPs inserted just before."""
    template = bass.Bass(target_bir_lowering=False).sync.nop(nofuse=True).ins
    ctr = 0
    for bb in nc.main_func.blocks:
        out = []
        changed = False
        for ins in bb.instructions:
            si = ins.sync_info
            if si is not None and si.on_wait and len(si.on_wait) > limit:
                waits = list(si.on_wait)
                for w in waits[:-limit]:
                    ctr += 1
                    nop = copy.deepcopy(template)
                    nop.name = f"I-wsplit-{ctr}"
                    nop.engine = ins.engine
                    nop.bass_nofuse = True
                    nop.sync_info = mybir.SyncInfo(on_wait=[w], on_update=[])
                    nc.register_instruction(nop, overwrite=True)
                    out.append(nop)
                ins.sync_info = mybir.SyncInfo(
                    on_wait=waits[-limit:], on_update=list(si.on_update)
                )
                changed = True
            out.append(ins)
        if changed:
            bb.instructions = out
    return ctr


# ---------------------------------------------------------------- routing
def _gate_logits(x, gate_w):
    # Match the reference's jax-f32 CPU matmul as closely as possible.
    try:
        import jax
        import jax.numpy as jnp

        cpu = jax.devices("cpu")[0]
        with jax.default_device(cpu):
            return np.asarray(jnp.matmul(jnp.asarray(x), jnp.asarray(gate_w)))
    except Exception:
        return (x @ gate_w).astype(np.float32)


def _route(x, gate_w, e_bias):
    logits = _gate_logits(x, gate_w)  # [T, E] f32
    scores = (1.0 / (1.0 + np.exp(-logits))).astype(np.float32)
    sfc = scores + e_bias[None, :]
    grp = sfc.reshape(T, N_GROUP, E // N_GROUP)
    group_scores = np.sort(grp, axis=-1)[:, :, -2:].sum(-1)  # [T, G]
    group_idx = np.argsort(-group_scores, axis=-1, kind="stable")[:, :TOPK_GROUP]
    group_mask = np.zeros((T, N_GROUP), bool)
    group_mask[np.arange(T)[:, None], group_idx] = True
    expert_mask = np.repeat(group_mask, E // N_GROUP, axis=1)
    masked = np.where(expert_mask, sfc, -np.inf)
    topk_idx = np.argsort(-masked, axis=-1, kind="stable")[:, :TOP_K]  # [T, 4]
    topk_w = np.take_along_axis(scores, topk_idx, axis=1)
    topk_w = topk_w / topk_w.sum(axis=1, keepdims=True)
    return topk_idx.astype(np.int64), topk_w.astype(np.float32)


# ---------------------------------------------------------------- program
_PROGRAM_CACHE = {}


def _emit_expert(nc, tc, pools, xt_h, w1_h, w2_h, wr_h, y_h, C, twoI, apply_wr):
    n_d = D // P  # 16 contraction chunks over D
    n_i = twoI // P  # gate_up output chunks
    n_h = n_i // 2  # h chunks (= I/128)
    chunks = [(o, min(NCHUNK, C - o)) for o in range(0, C, NCHUNK)]

    (xt_pool, w1_pool, w2_pool, g_pool, h_pool, y_pool, wr_pool, sg_pool,
     ps_gu, ps_dn) = pools

    # whole-expert X^T resident tile: [p, k(d-chunk), tok]; split the load
    # per d-chunk so the first matmuls start as soon as chunk 0 lands
    xt_t = xt_pool.tile([P, n_d, C], DT.bfloat16, name="xt")
    src = xt_h[:, :].rearrange("(k p) t -> p k t", p=P)
    for d in range(n_d):
        nc.sync.dma_start(xt_t[:, d, :], src[:, d, :])

    wr_t = None
    if apply_wr:
        wr_t = wr_pool.tile([P, C], DT.float32, name="wr")
        nc.sync.dma_start(wr_t[:], wr_h[:, :])

    g_tiles = {}
    h_tiles = {}
    for i in range(n_i):
        w1s = w1_pool.tile([P, n_d, P], DT.bfloat16, name="w1s")
        nc.sync.dma_start(w1s[:], w1_h[i])
        for ci, (off, sz) in enumerate(chunks):
            ps = ps_gu.tile([P, NCHUNK], DT.float32, name="psg")
            for d in range(n_d):
                nc.tensor.matmul(
                    ps[:, :sz],
                    w1s[:, d, :],
                    xt_t[:, d, off : off + sz],
                    start=(d == 0),
                    stop=(d == n_d - 1),
                )
            if i < n_h:
                sg = sg_pool.tile([P, NCHUNK], DT.float32, name="sg")
                nc.scalar.activation(
                    sg[:, :sz], ps[:, :sz], mybir.ActivationFunctionType.Sigmoid
                )
                gt = g_pool.tile([P, NCHUNK], DT.float32, name="gt")
                nc.vector.tensor_mul(gt[:, :sz], ps[:, :sz], sg[:, :sz])
                g_tiles[(i, ci)] = gt
            else:
                ht = h_pool.tile([P, NCHUNK], DT.bfloat16, name="ht")
                nc.vector.tensor_mul(
                    ht[:, :sz], ps[:, :sz], g_tiles[(i - n_h, ci)][:, :sz]
                )
                h_tiles[(i - n_h, ci)] = ht

    for d2 in range(D // P):
        w2s = w2_pool.tile([P, n_h, P], DT.bfloat16, name="w2s")
        nc.sync.dma_start(w2s[:], w2_h[d2])
        for ci, (off, sz) in enumerate(chunks):
            ps = ps_dn.tile([P, NCHUNK], DT.float32, name="psd")
            for hh in range(n_h):
                nc.tensor.matmul(
                    ps[:, :sz],
                    w2s[:, hh, :],
                    h_tiles[(hh, ci)][:, :sz],
                    start=(hh == 0),
                    stop=(hh == n_h - 1),
                )
            ys = y_pool.tile([P, NCHUNK], DT.float32, name="ys")
            if apply_wr:
                nc.vector.tensor_mul(ys[:, :sz], ps[:, :sz], wr_t[:, off : off + sz])
            else:
                nc.scalar.copy(ys[:, :sz], ps[:, :sz])
            nc.sync.dma_start(y_h[d2 * P : (d2 + 1) * P, off : off + sz], ys[:, :sz])


def _build_program(C1, C2):
    key = (C1, C2)
    if key in _PROGRAM_CACHE:
        return _PROGRAM_CACHE[key]

    nc = bass.Bass(target_bir_lowering=False)
    TS = T // N_CORES  # shared tokens per core

    xt1 = nc.dram_tensor("xt1", [D, C1], DT.bfloat16, kind="ExternalInput")
    xt2 = nc.dram_tensor("xt2", [D, C2], DT.bfloat16, kind="ExternalInput")
    xts = nc.dram_tensor("xts", [D, TS], DT.bfloat16, kind="ExternalInput")
    w1a = nc.dram_tensor("w1a", [2 * I // P, P, D // P, P], DT.bfloat16, kind="ExternalInput")
    w2a = nc.dram_tensor("w2a", [D // P, P, I // P, P], DT.bfloat16, kind="ExternalInput")
    w1b = nc.dram_tensor("w1b", [2 * I // P, P, D // P, P], DT.bfloat16, kind="ExternalInput")
    w2b = nc.dram_tensor("w2b", [D // P, P, I // P, P], DT.bfloat16, kind="ExternalInput")
    ws1 = nc.dram_tensor("ws1", [2 * SHARED_I // P, P, D // P, P], DT.bfloat16, kind="ExternalInput")
    ws2 = nc.dram_tensor("ws2", [D // P, P, SHARED_I // P, P], DT.bfloat16, kind="ExternalInput")
    wr1 = nc.dram_tensor("wr1", [P, C1], DT.float32, kind="ExternalInput")
    wr2 = nc.dram_tensor("wr2", [P, C2], DT.float32, kind="ExternalInput")
    y1 = nc.dram_tensor("y1", [D, C1], DT.float32, kind="ExternalOutput")
    y2 = nc.dram_tensor("y2", [D, C2], DT.float32, kind="ExternalOutput")
    ys = nc.dram_tensor("ys", [D, TS], DT.float32, kind="ExternalOutput")

    with tile.TileContext(nc) as tc:
        with (
            tc.tile_pool(name="xt", bufs=1) as xt_pool,
            tc.tile_pool(name="w1p", bufs=3) as w1_pool,
            tc.tile_pool(name="w2p", bufs=3) as w2_pool,
            tc.tile_pool(name="gp", bufs=24) as g_pool,
            tc.tile_pool(name="hp", bufs=24) as h_pool,
            tc.tile_pool(name="yp", bufs=4) as y_pool,
            tc.tile_pool(name="wrp", bufs=2) as wr_pool,
            tc.tile_pool(name="sgp", bufs=3) as sg_pool,
            tc.tile_pool(name="psgu", bufs=4, space="PSUM") as ps_gu,
            tc.tile_pool(name="psdn", bufs=4, space="PSUM") as ps_dn,
        ):
            pools = (xt_pool, w1_pool, w2_pool, g_pool, h_pool, y_pool, wr_pool,
                     sg_pool, ps_gu, ps_dn)
            _emit_expert(nc, tc, pools, xt1, w1a, w2a, wr1, y1, C1, 2 * I, True)
            _emit_expert(nc, tc, pools, xt2, w1b, w2b, wr2, y2, C2, 2 * I, True)
            _emit_expert(nc, tc, pools, xts, ws1, ws2, None, ys, T // N_CORES, 2 * SHARED_I, False)

    _split_excess_waits(nc, limit=1)
    _PROGRAM_CACHE[key] = nc
    return nc


# ---------------------------------------------------------------- packing
def _pack_w1(w, twoI):  # w [D, twoI] f32 -> [twoI/P, P, D/P, P] bf16
    n_d, n_i = D // P, twoI // P
    return np.ascontiguousarray(
        w.astype(BF16).reshape(n_d, P, n_i, P).transpose(2, 1, 0, 3)
    )


def _pack_w2(w, I_):  # w [I_, D] f32 -> [D/P, P, I_/P, P] bf16
    n_h, n_d2 = I_ // P, D // P
    return np.ascontiguousarray(
        w.astype(BF16).reshape(n_h, P, n_d2, P).transpose(2, 1, 0, 3)
    )


def _cap(n):
    # exact capacity (matmul free dim handles any size <= 512 per chunk);
    # keep a small floor so shapes stay sane for degenerate routings
    return max(P, int(n))


# ---------------------------------------------------------------- kernel
def _prepare(hidden_states, gate_w, e_bias, w_gate_up, w_down, ws_gate_up, ws_down):
    x = np.asarray(hidden_states, dtype=np.float32)
    topk_idx, topk_w = _route(x, np.asarray(gate_w), np.asarray(e_bias))

    # dispatch: token lists per expert, sorted-stable by expert id
    flat_e = topk_idx.ravel()
    order = np.argsort(flat_e, kind="stable")
    pair_tok = order // TOP_K
    pair_w = (topk_w.ravel()[order] * ROUTED_SCALE).astype(np.float32)
    counts = np.bincount(flat_e, minlength=E)
    starts = np.zeros(E + 1, np.int64)
    np.cumsum(counts, out=starts[1:])

    # expert -> core assignment: pair largest with smallest
    by_count = np.argsort(-counts, kind="stable")
    slotA = by_count[:N_CORES]
    slotB = by_count[E - 1 : N_CORES - 1 : -1]  # reversed smallest half
    C1 = _cap(counts[slotA].max())
    C2 = _cap(counts[slotB].max())

    nc = _build_program(C1, C2)

    x_bf = x.astype(BF16)
    xT_bf = np.ascontiguousarray(x_bf.T)  # [D, T]

    ws1_p = _pack_w1(np.asarray(ws_gate_up), 2 * SHARED_I)
    ws2_p = _pack_w2(np.asarray(ws_down), SHARED_I)
    w_gate_up = np.asarray(w_gate_up)
    w_down = np.asarray(w_down)

    TS = T // N_CORES
    in_maps = []
    core_info = []
    for c in range(N_CORES):
        eA, eB = int(slotA[c]), int(slotB[c])
        m = {}
        info = []
        for slot, e_id, C, xt_name, wr_name in (
            (0, eA, C1, "xt1", "wr1"),
            (1, eB, C2, "xt2", "wr2"),
        ):
            idx = pair_tok[starts[e_id] : starts[e_id + 1]]
            w = pair_w[starts[e_id] : starts[e_id + 1]]
            n_e = len(idx)
            idx_pad = np.zeros(C, np.int64)
            idx_pad[:n_e] = idx
            w_pad = np.zeros(C, np.float32)
            w_pad[:n_e] = w
            m[xt_name] = xT_bf[:, idx_pad]
            m[wr_name] = np.ascontiguousarray(
                np.broadcast_to(w_pad, (P, C))
            )
            info.append((idx, n_e))
        m["xts"] = np.ascontiguousarray(xT_bf[:, c * TS : (c + 1) * TS])
        m["w1a"] = _pack_w1(w_gate_up[eA], 2 * I)
        m["w2a"] = _pack_w2(w_down[eA], I)
        m["w1b"] = _pack_w1(w_gate_up[eB], 2 * I)
        m["w2b"] = _pack_w2(w_down[eB], I)
        m["ws1"] = ws1_p
        m["ws2"] = ws2_p
        in_maps.append(m)
        core_info.append(info)
    return nc, in_maps, core_info


def _combine(res_results, core_info):
    TS = T // N_CORES
    out = np.zeros((T, D), np.float32)
    for c in range(N_CORES):
        (idxA, nA), (idxB, nB) = core_info[c]
        out[idxA] += res_results[c]["y1"][:, :nA].T
        out[idxB] += res_results[c]["y2"][:, :nB].T
        out[c * TS : (c + 1) * TS] += res_results[c]["ys"].T
    return out


def kernel(hidden_states, gate_w, e_bias, w_gate_up, w_down, ws_gate_up, ws_down):
    nc, in_maps, core_info = _prepare(
        hidden_states, gate_w, e_bias, w_gate_up, w_down, ws_gate_up, ws_down
    )
    res = run_bass_kernel_spmd(nc, in_maps, list(range(N_CORES)))
    return _combine(res.results, core_info)



# revision 2
# speedup vs baseline: 1.2573x; 1.2573x over previous
"""DeepseekV2 MoE layer on 8 Trainium2 NeuronCores.

Strategy (expert-parallel, per the sharding hint):
  - Router gate + grouped top-k computed on host (0.03% of module FLOPs);
    it determines the dispatch, which IS the input sharding.
  - 16 routed experts paired big-count-with-small-count onto 8 cores
    (2 experts per core, token lists gathered host-side, padded to a
    shared per-slot capacity so all cores run one SPMD program).
  - Shared-expert MLP is data-parallel over tokens: each core runs
    T/8 = 512 tokens through the full shared MLP.
  - All matmuls run as fp8(e4m3) DoubleRow pairs with 3-term hi/lo error
    compensation: for every operand pair (W, X), W = W_hi + W_lo and
    X = X_hi + X_lo in scaled e4m3; the product is computed as
    W_hi.X_hi + (W_hi.X_lo + W_lo.X_hi), dropping the negligible
    W_lo.X_lo term. Each DoubleRow instruction carries two K=128
    products, so a K=256 contraction costs 3 instructions vs 2 for
    bf16 while retaining (slightly better than) bf16 accuracy.
  - f32 PSUM accumulation; bf16 outputs (combined in f32 on host).
"""

import sys

sys.path.insert(0, "/opt/trn_rl_repo")

import copy

import ml_dtypes
import numpy as np

import concourse.bass as bass
import concourse.mybir as mybir
import concourse.tile as tile
from concourse.bass_utils import run_bass_kernel_spmd

DT = mybir.dt
F8 = ml_dtypes.float8_e4m3
BF16 = ml_dtypes.bfloat16
DR = mybir.MatmulPerfMode.DoubleRow

T, D, E, I = 4096, 2048, 16, 1024
TOP_K, N_GROUP, TOPK_GROUP = 4, 4, 2
ROUTED_SCALE = 2.5
SHARED_I = 2048
N_CORES = 8
P = 128
NCH = 256  # token chunk (DoubleRow moving free = 2*NCH = 512 max)

SX = 16.0  # x scale into e4m3
SW = 512.0  # weight scale into e4m3
SH = 8.0  # h scale into e4m3
CU = SH / (SX * SW * SX * SW)  # ps_u -> u*SH/(SX*SW)
CY = 1.0 / (SH * SW)  # down psum descale


# ---------------------------------------------------------------- wait split
def _split_excess_waits(nc, limit=1):
    """This walrus build rejects >1 sync-wait command per instruction.
    Move excess waits onto fresh same-engine NOPs inserted just before."""
    template = bass.Bass(target_bir_lowering=False).sync.nop(nofuse=True).ins
    ctr = 0
    for bb in nc.main_func.blocks:
        out = []
        changed = False
        for ins in bb.instructions:
            si = ins.sync_info
            if si is not None and si.on_wait and len(si.on_wait) > limit:
                waits = list(si.on_wait)
                for w in waits[:-limit]:
                    ctr += 1
                    nop = copy.deepcopy(template)
                    nop.name = f"I-wsplit-{ctr}"
                    nop.engine = ins.engine
                    nop.bass_nofuse = True
                    nop.sync_info = mybir.SyncInfo(on_wait=[w], on_update=[])
                    nc.register_instruction(nop, overwrite=True)
                    out.append(nop)
                ins.sync_info = mybir.SyncInfo(
                    on_wait=waits[-limit:], on_update=list(si.on_update)
                )
                changed = True
            out.append(ins)
        if changed:
            bb.instructions = out
    return ctr


# ---------------------------------------------------------------- routing
def _gate_logits(x, gate_w):
    # Match the reference's jax-f32 CPU matmul as closely as possible.
    try:
        import jax
        import jax.numpy as jnp

        cpu = jax.devices("cpu")[0]
        with jax.default_device(cpu):
            return np.asarray(jnp.matmul(jnp.asarray(x), jnp.asarray(gate_w)))
    except Exception:
        return (x @ gate_w).astype(np.float32)


def _route(x, gate_w, e_bias):
    logits = _gate_logits(x, gate_w)  # [T, E] f32
    scores = (1.0 / (1.0 + np.exp(-logits))).astype(np.float32)
    sfc = scores + e_bias[None, :]
    grp = sfc.reshape(T, N_GROUP, E // N_GROUP)
    group_scores = np.sort(grp, axis=-1)[:, :, -2:].sum(-1)  # [T, G]
    group_idx = np.argsort(-group_scores, axis=-1, kind="stable")[:, :TOPK_GROUP]
    group_mask = np.zeros((T, N_GROUP), bool)
    group_mask[np.arange(T)[:, None], group_idx] = True
    expert_mask = np.repeat(group_mask, E // N_GROUP, axis=1)
    masked = np.where(expert_mask, sfc, -np.inf)
    topk_idx = np.argsort(-masked, axis=-1, kind="stable")[:, :TOP_K]  # [T, 4]
    topk_w = np.take_along_axis(scores, topk_idx, axis=1)
    topk_w = topk_w / topk_w.sum(axis=1, keepdims=True)
    return topk_idx.astype(np.int64), topk_w.astype(np.float32)


# ---------------------------------------------------------------- program
_PROGRAM_CACHE = {}


def _mm3(nc, ps, wt, xt, nk, tok, sz, first, last):
    """3-term compensated fp8 DoubleRow contraction over nk k-slices of 128.

    wt: stationary tile [P, nk, 2, P] with slot0=hi, slot1=lo.
    xt: moving tile [P, nk, 2, C] with slot0=lo, slot1=hi.
    ps: psum [P, NCH] (use [:, :sz]); tok = token offset into xt.
    """
    # hi*hi over k-slice pairs
    for j in range(nk // 2):
        nc.tensor.matmul(
            ps[:, :sz],
            wt[:, 2 * j : 2 * j + 2, 0, :],
            xt[:, 2 * j : 2 * j + 2, 1, tok : tok + sz],
            start=(first and j == 0),
            stop=False,
            perf_mode=DR,
        )
    # cross terms: (w_hi, w_lo) x (x_lo, x_hi) per k-slice
    for k in range(nk):
        nc.tensor.matmul(
            ps[:, :sz],
            wt[:, k, :, :],
            xt[:, k, :, tok : tok + sz],
            start=False,
            stop=(last and k == nk - 1),
            perf_mode=DR,
        )


def _emit_expert(nc, tc, pools, xt_h, w1_h, w2_h, wr_h, y_h, C, twoI, apply_wr):
    n_d = D // P  # 16 contraction slices over D
    n_i = twoI // P  # gate_up output tiles
    n_h = n_i // 2  # h tiles (= I_/128)
    chunks = [(o, min(NCH, C - o)) for o in range(0, C, NCH)]

    (xt_pool, w1_pool, w2_pool, g_pool, h_pool, y_pool, wr_pool, sg_pool,
     tmp_pool, ps_gu, ps_dn) = pools

    # whole-expert X tile [P, k-slice, (lo,hi), tok]; split the load per
    # k-slice so the first matmuls start as soon as slice 0 lands
    xt_t = xt_pool.tile([P, n_d, 2, C], DT.float8e4, name="xt")
    for d in range(n_d):
        nc.sync.dma_start(xt_t[:, d, :, :], xt_h[:, d])

    wr_t = None
    if apply_wr:
        wr_t = wr_pool.tile([P, C], DT.float32, name="wr")
        nc.sync.dma_start(wr_t[:], wr_h[:, :])

    h_t = h_pool.tile([P, n_h, 2, C], DT.float8e4, name="hil")

    # gate_up: process (gate i, up i+n_h) pairs so gate tiles die quickly
    for ih in range(n_h):
        gt_tiles = {}
        for half, i in ((0, ih), (1, ih + n_h)):
            w1s = w1_pool.tile([P, n_d, 2, P], DT.float8e4, name="w1s")
            nc.sync.dma_start(w1s[:], w1_h[i])
            for ci, (off, sz) in enumerate(chunks):
                ps = ps_gu.tile([P, NCH], DT.float32, name="psg")
                _mm3(nc, ps, w1s, xt_t, n_d, off, sz, True, True)
                if half == 0:
                    sg = sg_pool.tile([P, NCH], DT.float32, name="sg")
                    nc.scalar.activation(
                        sg[:, :sz], ps[:, :sz],
                        mybir.ActivationFunctionType.Sigmoid,
                        scale=1.0 / (SX * SW),
                    )
                    gt = g_pool.tile([P, NCH], DT.float32, name="gt")
                    nc.vector.tensor_mul(gt[:, :sz], ps[:, :sz], sg[:, :sz])
                    gt_tiles[ci] = gt
                else:
                    us = tmp_pool.tile([P, NCH], DT.float32, name="us")
                    nc.vector.tensor_scalar_mul(us[:, :sz], ps[:, :sz], CU)
                    th = tmp_pool.tile([P, NCH], DT.float32, name="th")
                    nc.vector.tensor_mul(th[:, :sz], gt_tiles[ci][:, :sz], us[:, :sz])
                    nc.scalar.copy(h_t[:, ih, 1, off : off + sz], th[:, :sz])
                    df = tmp_pool.tile([P, NCH], DT.float32, name="df")
                    nc.vector.tensor_sub(
                        df[:, :sz], th[:, :sz], h_t[:, ih, 1, off : off + sz]
                    )
                    nc.scalar.copy(h_t[:, ih, 0, off : off + sz], df[:, :sz])

    # down projection
    for d2 in range(D // P):
        w2s = w2_pool.tile([P, n_h, 2, P], DT.float8e4, name="w2s")
        nc.sync.dma_start(w2s[:], w2_h[d2])
        for ci, (off, sz) in enumerate(chunks):
            ps = ps_dn.tile([P, NCH], DT.float32, name="psd")
            _mm3(nc, ps, w2s, h_t, n_h, off, sz, True, True)
            ys = y_pool.tile([P, NCH], DT.bfloat16, name="ys")
            if apply_wr:
                nc.vector.tensor_mul(ys[:, :sz], ps[:, :sz], wr_t[:, off : off + sz])
            else:
                nc.scalar.mul(ys[:, :sz], ps[:, :sz], CY)
            nc.sync.dma_start(y_h[d2 * P : (d2 + 1) * P, off : off + sz], ys[:, :sz])


def _build_program(C1, C2):
    key = (C1, C2)
    if key in _PROGRAM_CACHE:
        return _PROGRAM_CACHE[key]

    nc = bass.Bass(target_bir_lowering=False)
    TS = T // N_CORES  # shared tokens per core
    n_d = D // P

    xt1 = nc.dram_tensor("xt1", [P, n_d, 2, C1], DT.float8e4, kind="ExternalInput")
    xt2 = nc.dram_tensor("xt2", [P, n_d, 2, C2], DT.float8e4, kind="ExternalInput")
    xts = nc.dram_tensor("xts", [P, n_d, 2, TS], DT.float8e4, kind="ExternalInput")
    w1a = nc.dram_tensor("w1a", [2 * I // P, P, n_d, 2, P], DT.float8e4, kind="ExternalInput")
    w2a = nc.dram_tensor("w2a", [D // P, P, I // P, 2, P], DT.float8e4, kind="ExternalInput")
    w1b = nc.dram_tensor("w1b", [2 * I // P, P, n_d, 2, P], DT.float8e4, kind="ExternalInput")
    w2b = nc.dram_tensor("w2b", [D // P, P, I // P, 2, P], DT.float8e4, kind="ExternalInput")
    ws1 = nc.dram_tensor("ws1", [2 * SHARED_I // P, P, n_d, 2, P], DT.float8e4, kind="ExternalInput")
    ws2 = nc.dram_tensor("ws2", [D // P, P, SHARED_I // P, 2, P], DT.float8e4, kind="ExternalInput")
    wr1 = nc.dram_tensor("wr1", [P, C1], DT.float32, kind="ExternalInput")
    wr2 = nc.dram_tensor("wr2", [P, C2], DT.float32, kind="ExternalInput")
    y1 = nc.dram_tensor("y1", [D, C1], DT.bfloat16, kind="ExternalOutput")
    y2 = nc.dram_tensor("y2", [D, C2], DT.bfloat16, kind="ExternalOutput")
    ys = nc.dram_tensor("ys", [D, TS], DT.bfloat16, kind="ExternalOutput")

    with tile.TileContext(nc) as tc:
        with (
            tc.tile_pool(name="xt", bufs=2) as xt_pool,
            tc.tile_pool(name="w1p", bufs=3) as w1_pool,
            tc.tile_pool(name="w2p", bufs=3) as w2_pool,
            tc.tile_pool(name="gp", bufs=8) as g_pool,
            tc.tile_pool(name="hp", bufs=2) as h_pool,
            tc.tile_pool(name="yp", bufs=4) as y_pool,
            tc.tile_pool(name="wrp", bufs=2) as wr_pool,
            tc.tile_pool(name="sgp", bufs=3) as sg_pool,
            tc.tile_pool(name="tmp", bufs=6) as tmp_pool,
            tc.tile_pool(name="psgu", bufs=4, space="PSUM") as ps_gu,
            tc.tile_pool(name="psdn", bufs=4, space="PSUM") as ps_dn,
        ):
            pools = (xt_pool, w1_pool, w2_pool, g_pool, h_pool, y_pool, wr_pool,
                     sg_pool, tmp_pool, ps_gu, ps_dn)
            _emit_expert(nc, tc, pools, xt1, w1a, w2a, wr1, y1, C1, 2 * I, True)
            _emit_expert(nc, tc, pools, xt2, w1b, w2b, wr2, y2, C2, 2 * I, True)
            _emit_expert(nc, tc, pools, xts, ws1, ws2, None, ys, TS, 2 * SHARED_I, False)

    _split_excess_waits(nc, limit=1)
    _PROGRAM_CACHE[key] = nc
    return nc


# ---------------------------------------------------------------- packing
def _hi_lo(a, scale):
    s = (a * scale).astype(np.float32)
    hi = s.astype(F8)
    lo = (s - hi.astype(np.float32)).astype(F8)
    return hi, lo


def _pack_w(w, scale):
    """w [K, F] f32 -> [F/P, P(k-in-slice), K/P, 2(hi,lo), P(feat)] e4m3."""
    K, F = w.shape
    n_k, n_f = K // P, F // P
    hi, lo = _hi_lo(w, scale)

    def arr(a):
        return a.reshape(n_k, P, n_f, P).transpose(2, 1, 0, 3)

    out = np.empty((n_f, P, n_k, 2, P), F8)
    out[:, :, :, 0, :] = arr(hi)
    out[:, :, :, 1, :] = arr(lo)
    return np.ascontiguousarray(out)


def _pack_x(xhiT, xloT, cols):
    """xhiT/xloT [D, T] e4m3 + column index -> [P, D/P, 2(lo,hi), C]."""
    n_d = D // P
    C = len(cols)
    out = np.empty((P, n_d, 2, C), F8)
    out[:, :, 0, :] = xloT[:, cols].reshape(n_d, P, C).transpose(1, 0, 2)
    out[:, :, 1, :] = xhiT[:, cols].reshape(n_d, P, C).transpose(1, 0, 2)
    return np.ascontiguousarray(out)


def _cap(n):
    # exact capacity; keep a small floor so degenerate routings stay sane
    return max(P, int(n))


# ---------------------------------------------------------------- kernel
def _prepare(hidden_states, gate_w, e_bias, w_gate_up, w_down, ws_gate_up, ws_down):
    x = np.asarray(hidden_states, dtype=np.float32)
    topk_idx, topk_w = _route(x, np.asarray(gate_w), np.asarray(e_bias))

    # dispatch: token lists per expert, sorted-stable by expert id
    flat_e = topk_idx.ravel()
    order = np.argsort(flat_e, kind="stable")
    pair_tok = order // TOP_K
    pair_w = (topk_w.ravel()[order] * ROUTED_SCALE).astype(np.float32)
    counts = np.bincount(flat_e, minlength=E)
    starts = np.zeros(E + 1, np.int64)
    np.cumsum(counts, out=starts[1:])

    # expert -> core assignment: pair largest with smallest
    by_count = np.argsort(-counts, kind="stable")
    slotA = by_count[:N_CORES]
    slotB = by_count[E - 1 : N_CORES - 1 : -1]  # reversed smallest half
    C1 = _cap(counts[slotA].max())
    C2 = _cap(counts[slotB].max())

    nc = _build_program(C1, C2)

    xhi, xlo = _hi_lo(x, SX)  # [T, D] e4m3
    xhiT = np.ascontiguousarray(xhi.T)  # [D, T]
    xloT = np.ascontiguousarray(xlo.T)

    ws1_p = _pack_w(np.asarray(ws_gate_up), SW)
    ws2_p = _pack_w(np.asarray(ws_down), SW)
    w_gate_up = np.asarray(w_gate_up)
    w_down = np.asarray(w_down)

    TS = T // N_CORES
    in_maps = []
    core_info = []
    for c in range(N_CORES):
        eA, eB = int(slotA[c]), int(slotB[c])
        m = {}
        info = []
        for slot, e_id, C, xt_name, wr_name in (
            (0, eA, C1, "xt1", "wr1"),
            (1, eB, C2, "xt2", "wr2"),
        ):
            idx = pair_tok[starts[e_id] : starts[e_id + 1]]
            w = pair_w[starts[e_id] : starts[e_id + 1]]
            n_e = len(idx)
            idx_pad = np.zeros(C, np.int64)
            idx_pad[:n_e] = idx
            w_pad = np.zeros(C, np.float32)
            w_pad[:n_e] = w * CY
            m[xt_name] = _pack_x(xhiT, xloT, idx_pad)
            m[wr_name] = np.ascontiguousarray(np.broadcast_to(w_pad, (P, C)))
            info.append((idx, n_e))
        m["xts"] = _pack_x(xhiT, xloT, np.arange(c * TS, (c + 1) * TS))
        m["w1a"] = _pack_w(w_gate_up[eA], SW)
        m["w2a"] = _pack_w(w_down[eA], SW)
        m["w1b"] = _pack_w(w_gate_up[eB], SW)
        m["w2b"] = _pack_w(w_down[eB], SW)
        m["ws1"] = ws1_p
        m["ws2"] = ws2_p
        in_maps.append(m)
        core_info.append(info)
    return nc, in_maps, core_info


def _combine(res_results, core_info):
    TS = T // N_CORES
    out = np.zeros((T, D), np.float32)
    for c in range(N_CORES):
        (idxA, nA), (idxB, nB) = core_info[c]
        out[idxA] += res_results[c]["y1"][:, :nA].astype(np.float32).T
        out[idxB] += res_results[c]["y2"][:, :nB].astype(np.float32).T
        out[c * TS : (c + 1) * TS] += res_results[c]["ys"].astype(np.float32).T
    return out


def kernel(hidden_states, gate_w, e_bias, w_gate_up, w_down, ws_gate_up, ws_down):
    nc, in_maps, core_info = _prepare(
        hidden_states, gate_w, e_bias, w_gate_up, w_down, ws_gate_up, ws_down
    )
    res = run_bass_kernel_spmd(nc, in_maps, list(range(N_CORES)))
    return _combine(res.results, core_info)


# revision 30
# speedup vs baseline: 1.3597x; 1.0814x over previous
"""DeepseekV2 MoE layer on 8 Trainium2 NeuronCores.

Strategy (expert-parallel, per the sharding hint):
  - Router gate + grouped top-k computed on host (0.03% of module FLOPs);
    it determines the dispatch, which IS the input sharding.
  - 16 routed experts paired big-count-with-small-count onto 8 cores
    (2 experts per core, token lists gathered host-side, padded to a
    shared per-slot capacity so all cores run one SPMD program).
  - Shared-expert MLP is data-parallel over tokens: each core runs
    T/8 = 512 tokens through the full shared MLP.
  - All matmuls run as fp8(e4m3) DoubleRow pairs with 3-term hi/lo error
    compensation: for every operand pair (W, X), W = W_hi + W_lo and
    X = X_hi + X_lo in scaled e4m3; the product is computed as
    W_hi.X_hi + (W_hi.X_lo + W_lo.X_hi), dropping the negligible
    W_lo.X_lo term. Each DoubleRow instruction carries two K=128
    products, so a K=256 contraction costs 3 instructions vs 2 for
    bf16 while retaining (slightly better than) bf16 accuracy.
  - f32 PSUM accumulation; bf16 outputs (combined in f32 on host).
"""

import sys

sys.path.insert(0, "/opt/trn_rl_repo")

import copy

import ml_dtypes
import numpy as np

import concourse.bass as bass
import concourse.mybir as mybir
import concourse.tile as tile
from concourse.bass_utils import run_bass_kernel_spmd

DT = mybir.dt
F8 = ml_dtypes.float8_e4m3
BF16 = ml_dtypes.bfloat16
DR = mybir.MatmulPerfMode.DoubleRow

T, D, E, I = 4096, 2048, 16, 1024
TOP_K, N_GROUP, TOPK_GROUP = 4, 4, 2
ROUTED_SCALE = 2.5
SHARED_I = 2048
N_CORES = 8
P = 128
NCH = 256  # token chunk (DoubleRow moving free = 2*NCH = 512 max)

SX = 16.0  # x scale into e4m3
SW = 512.0  # weight scale into e4m3
SH = 8.0  # h scale into e4m3
CU = SH / (SX * SW * SX * SW)  # ps_u -> u*SH/(SX*SW)
CY = 1.0 / (SH * SW)  # down psum descale


# ---------------------------------------------------------------- wait split
def _split_excess_waits(nc, limit=1):
    """This walrus build rejects >1 sync-wait command per instruction.
    Move excess waits onto fresh same-engine NOPs inserted just before."""
    template = bass.Bass(target_bir_lowering=False).sync.nop(nofuse=True).ins
    ctr = 0
    for bb in nc.main_func.blocks:
        out = []
        changed = False
        for ins in bb.instructions:
            si = ins.sync_info
            if si is not None and si.on_wait and len(si.on_wait) > limit:
                waits = list(si.on_wait)
                for w in waits[:-limit]:
                    ctr += 1
                    nop = copy.deepcopy(template)
                    nop.name = f"I-wsplit-{ctr}"
                    nop.engine = ins.engine
                    nop.bass_nofuse = True
                    nop.sync_info = mybir.SyncInfo(on_wait=[w], on_update=[])
                    nc.register_instruction(nop, overwrite=True)
                    out.append(nop)
                ins.sync_info = mybir.SyncInfo(
                    on_wait=waits[-limit:], on_update=list(si.on_update)
                )
                changed = True
            out.append(ins)
        if changed:
            bb.instructions = out
    return ctr


# ---------------------------------------------------------------- routing
def _gate_logits(x, gate_w):
    # Match the reference's jax-f32 CPU matmul as closely as possible.
    try:
        import jax
        import jax.numpy as jnp

        cpu = jax.devices("cpu")[0]
        with jax.default_device(cpu):
            return np.asarray(jnp.matmul(jnp.asarray(x), jnp.asarray(gate_w)))
    except Exception:
        return (x @ gate_w).astype(np.float32)


def _route(x, gate_w, e_bias):
    logits = _gate_logits(x, gate_w)  # [T, E] f32
    scores = (1.0 / (1.0 + np.exp(-logits))).astype(np.float32)
    sfc = scores + e_bias[None, :]
    grp = sfc.reshape(T, N_GROUP, E // N_GROUP)
    group_scores = np.sort(grp, axis=-1)[:, :, -2:].sum(-1)  # [T, G]
    group_idx = np.argsort(-group_scores, axis=-1, kind="stable")[:, :TOPK_GROUP]
    group_mask = np.zeros((T, N_GROUP), bool)
    group_mask[np.arange(T)[:, None], group_idx] = True
    expert_mask = np.repeat(group_mask, E // N_GROUP, axis=1)
    masked = np.where(expert_mask, sfc, -np.inf)
    topk_idx = np.argsort(-masked, axis=-1, kind="stable")[:, :TOP_K]  # [T, 4]
    topk_w = np.take_along_axis(scores, topk_idx, axis=1)
    topk_w = topk_w / topk_w.sum(axis=1, keepdims=True)
    return topk_idx.astype(np.int64), topk_w.astype(np.float32)


# ---------------------------------------------------------------- program
_PROGRAM_CACHE = {}


def _mm3(nc, ps, wt, xt, nk, tok, sz, first, last):
    """3-term compensated fp8 DoubleRow contraction over nk k-slices of 128.

    wt: stationary tile [P, nk, 2, P] with slot0=hi, slot1=lo.
    xt: moving tile [P, nk, 2, C] with slot0=lo, slot1=hi.
    ps: psum [P, NCH] (use [:, :sz]); tok = token offset into xt.
    """
    # hi*hi over k-slice pairs
    for j in range(nk // 2):
        nc.tensor.matmul(
            ps[:, :sz],
            wt[:, 2 * j : 2 * j + 2, 0, :],
            xt[:, 2 * j : 2 * j + 2, 1, tok : tok + sz],
            start=(first and j == 0),
            stop=False,
            perf_mode=DR,
        )
    # cross terms: (w_hi, w_lo) x (x_lo, x_hi) per k-slice
    for k in range(nk):
        nc.tensor.matmul(
            ps[:, :sz],
            wt[:, k, :, :],
            xt[:, k, :, tok : tok + sz],
            start=False,
            stop=(last and k == nk - 1),
            perf_mode=DR,
        )


def _emit_expert(nc, tc, pools, xt_h, w1_h, w2_h, wr_h, y_h, C, twoI, apply_wr,
                 bulk_q=False, first=False):
    n_d = D // P  # 16 contraction slices over D
    n_i = twoI // P  # gate_up output tiles
    n_h = n_i // 2  # h tiles (= I_/128)
    chunks = [(o, min(NCH, C - o)) for o in range(0, C, NCH)]

    (xt_pool, w1_pool, w2_pool, g_pool, h_pool, y_pool, wr_pool, sg_pool,
     tmp_pool, ps_gu, ps_dn) = pools

    # w1 slices consumed as (gate i, up i+n_h) pairs; prefetch 2 ahead so
    # slice loads are never just-in-time
    w1_order = []
    for ih in range(n_h):
        w1_order += [ih, ih + n_h]

    def load_w1(i, q=None):
        t = w1_pool.tile([P, n_d, 2, P], DT.float8e4, name="w1s")
        (q or nc.sync).dma_start(t[:], w1_h[i])
        return t

    # preload the first weight slices on the Pool queue: it idles between
    # the (deferred) x bulk loads, so these land without queueing behind the
    # previous expert's weight stream on SP
    n_pre = 6 if first else 3
    w1_tiles = {j: load_w1(w1_order[j], nc.gpsimd) for j in range(n_pre)}

    # whole-expert X tile [P, k-slice, (lo,hi), tok].  First expert: 4 chunky
    # loads (SP-issue rate is the cold-start limiter).  Later experts: per-d
    # slices on the Pool queue, so each transfer is short and never
    # head-of-line-blocks the latency-critical weight-slice stream on the
    # shared DMA engines.
    xt_q = nc.gpsimd if bulk_q else nc.sync
    xt_t = xt_pool.tile([P, n_d, 2, C], DT.float8e4, name="xt")
    step = 4 if first else 1
    for d in range(0, n_d, step):
        xt_q.dma_start(xt_t[:, d : d + step, :, :], xt_h[:, d : d + step])

    wr_t = None
    if apply_wr:
        wr_t = wr_pool.tile([P, C], DT.float32, name="wr")
        xt_q.dma_start(wr_t[:], wr_h[:, :])

    h_t = h_pool.tile([P, n_h, 2, C], DT.float8e4, name="hil")

    # gate_up: process (gate i, up i+n_h) pairs so gate tiles die quickly
    for ih in range(n_h):
        gt_tiles = {}
        for half, i in ((0, ih), (1, ih + n_h)):
            idx = 2 * ih + half
            w1s = w1_tiles.pop(idx)
            if idx + n_pre < len(w1_order):
                w1_tiles[idx + n_pre] = load_w1(w1_order[idx + n_pre])
            for ci, (off, sz) in enumerate(chunks):
                ps = ps_gu.tile([P, NCH], DT.float32, name="psg")
                _mm3(nc, ps, w1s, xt_t, n_d, off, sz, True, True)
                if half == 0:
                    sg = sg_pool.tile([P, NCH], DT.float32, name="sg")
                    nc.scalar.activation(
                        sg[:, :sz], ps[:, :sz],
                        mybir.ActivationFunctionType.Sigmoid,
                        scale=1.0 / (SX * SW),
                    )
                    gt = g_pool.tile([P, NCH], DT.float32, name="gt")
                    nc.vector.tensor_mul(gt[:, :sz], ps[:, :sz], sg[:, :sz])
                    gt_tiles[ci] = gt
                else:
                    us = tmp_pool.tile([P, NCH], DT.float32, name="us")
                    nc.vector.tensor_scalar_mul(us[:, :sz], ps[:, :sz], CU)
                    th = tmp_pool.tile([P, NCH], DT.float32, name="th")
                    nc.vector.tensor_mul(th[:, :sz], gt_tiles[ci][:, :sz], us[:, :sz])
                    nc.scalar.copy(h_t[:, ih, 1, off : off + sz], th[:, :sz])
                    df = tmp_pool.tile([P, NCH], DT.float32, name="df")
                    nc.vector.tensor_sub(
                        df[:, :sz], th[:, :sz], h_t[:, ih, 1, off : off + sz]
                    )
                    nc.scalar.copy(h_t[:, ih, 0, off : off + sz], df[:, :sz])

    # down projection (w2 slices prefetched 2 ahead)
    def load_w2(d2):
        t = w2_pool.tile([P, n_h, 2, P], DT.float8e4, name="w2s")
        nc.sync.dma_start(t[:], w2_h[d2])
        return t

    w2_tiles = {0: load_w2(0), 1: load_w2(1)}
    for d2 in range(D // P):
        w2s = w2_tiles.pop(d2)
        if d2 + 2 < D // P:
            w2_tiles[d2 + 2] = load_w2(d2 + 2)
        ys = y_pool.tile([P, C], DT.bfloat16, name="ys")
        y_q = nc.scalar if d2 % 2 else nc.sync
        half = len(chunks) // 2
        for ci, (off, sz) in enumerate(chunks):
            ps = ps_dn.tile([P, NCH], DT.float32, name="psd")
            _mm3(nc, ps, w2s, h_t, n_h, off, sz, True, True)
            if apply_wr:
                nc.vector.tensor_mul(ys[:, off : off + sz], ps[:, :sz], wr_t[:, off : off + sz])
            else:
                nc.scalar.mul(ys[:, off : off + sz], ps[:, :sz], CY)
            if len(chunks) > 2 and ci == half - 1:
                mid = chunks[half][0]
                y_q.dma_start(y_h[d2 * P : (d2 + 1) * P, :mid], ys[:, :mid])
        lo = chunks[half][0] if len(chunks) > 2 else 0
        y_q.dma_start(y_h[d2 * P : (d2 + 1) * P, lo:], ys[:, lo:])


def _build_program(C1, C2):
    key = (C1, C2)
    if key in _PROGRAM_CACHE:
        return _PROGRAM_CACHE[key]

    nc = bass.Bass(target_bir_lowering=False)
    TS = T // N_CORES  # shared tokens per core
    n_d = D // P

    xt1 = nc.dram_tensor("xt1", [P, n_d, 2, C1], DT.float8e4, kind="ExternalInput")
    xt2 = nc.dram_tensor("xt2", [P, n_d, 2, C2], DT.float8e4, kind="ExternalInput")
    xts = nc.dram_tensor("xts", [P, n_d, 2, TS], DT.float8e4, kind="ExternalInput")
    w1a = nc.dram_tensor("w1a", [2 * I // P, P, n_d, 2, P], DT.float8e4, kind="ExternalInput")
    w2a = nc.dram_tensor("w2a", [D // P, P, I // P, 2, P], DT.float8e4, kind="ExternalInput")
    w1b = nc.dram_tensor("w1b", [2 * I // P, P, n_d, 2, P], DT.float8e4, kind="ExternalInput")
    w2b = nc.dram_tensor("w2b", [D // P, P, I // P, 2, P], DT.float8e4, kind="ExternalInput")
    ws1 = nc.dram_tensor("ws1", [2 * SHARED_I // P, P, n_d, 2, P], DT.float8e4, kind="ExternalInput")
    ws2 = nc.dram_tensor("ws2", [D // P, P, SHARED_I // P, 2, P], DT.float8e4, kind="ExternalInput")
    wr1 = nc.dram_tensor("wr1", [P, C1], DT.float32, kind="ExternalInput")
    wr2 = nc.dram_tensor("wr2", [P, C2], DT.float32, kind="ExternalInput")
    y1 = nc.dram_tensor("y1", [D, C1], DT.bfloat16, kind="ExternalOutput")
    y2 = nc.dram_tensor("y2", [D, C2], DT.bfloat16, kind="ExternalOutput")
    ys = nc.dram_tensor("ys", [D, TS], DT.bfloat16, kind="ExternalOutput")

    with tile.TileContext(nc) as tc:
        with (
            tc.tile_pool(name="xt", bufs=1) as xt_pool,
            tc.tile_pool(name="w1p", bufs=7) as w1_pool,
            tc.tile_pool(name="w2p", bufs=4) as w2_pool,
            tc.tile_pool(name="gp", bufs=8) as g_pool,
            tc.tile_pool(name="hp", bufs=2) as h_pool,
            tc.tile_pool(name="yp", bufs=3) as y_pool,
            tc.tile_pool(name="wrp", bufs=2) as wr_pool,
            tc.tile_pool(name="sgp", bufs=3) as sg_pool,
            tc.tile_pool(name="tmp", bufs=4) as tmp_pool,
            tc.tile_pool(name="psgu", bufs=4, space="PSUM") as ps_gu,
            tc.tile_pool(name="psdn", bufs=4, space="PSUM") as ps_dn,
        ):
            pools = (xt_pool, w1_pool, w2_pool, g_pool, h_pool, y_pool, wr_pool,
                     sg_pool, tmp_pool, ps_gu, ps_dn)
            # shared first: its small x-load makes the cold-start short, and
            # the routed experts' larger input streams prefetch underneath it
            _emit_expert(nc, tc, pools, xts, ws1, ws2, None, ys, TS, 2 * SHARED_I, False,
                         first=True)
            _emit_expert(nc, tc, pools, xt1, w1a, w2a, wr1, y1, C1, 2 * I, True,
                         bulk_q=True)
            _emit_expert(nc, tc, pools, xt2, w1b, w2b, wr2, y2, C2, 2 * I, True,
                         bulk_q=True)

    _split_excess_waits(nc, limit=1)
    _PROGRAM_CACHE[key] = nc
    return nc


# ---------------------------------------------------------------- packing
def _hi_lo(a, scale):
    s = (a * scale).astype(np.float32)
    hi = s.astype(F8)
    lo = (s - hi.astype(np.float32)).astype(F8)
    return hi, lo


def _pack_w(w, scale):
    """w [K, F] f32 -> [F/P, P(k-in-slice), K/P, 2(hi,lo), P(feat)] e4m3."""
    K, F = w.shape
    n_k, n_f = K // P, F // P
    hi, lo = _hi_lo(w, scale)

    def arr(a):
        return a.reshape(n_k, P, n_f, P).transpose(2, 1, 0, 3)

    out = np.empty((n_f, P, n_k, 2, P), F8)
    out[:, :, :, 0, :] = arr(hi)
    out[:, :, :, 1, :] = arr(lo)
    return np.ascontiguousarray(out)


def _pack_x(xhiT, xloT, cols):
    """xhiT/xloT [D, T] e4m3 + column index -> [P, D/P, 2(lo,hi), C]."""
    n_d = D // P
    C = len(cols)
    out = np.empty((P, n_d, 2, C), F8)
    out[:, :, 0, :] = xloT[:, cols].reshape(n_d, P, C).transpose(1, 0, 2)
    out[:, :, 1, :] = xhiT[:, cols].reshape(n_d, P, C).transpose(1, 0, 2)
    return np.ascontiguousarray(out)


def _cap(n):
    # exact capacity; keep a small floor so degenerate routings stay sane
    return max(P, int(n))


# ---------------------------------------------------------------- kernel
def _prepare(hidden_states, gate_w, e_bias, w_gate_up, w_down, ws_gate_up, ws_down):
    x = np.asarray(hidden_states, dtype=np.float32)
    topk_idx, topk_w = _route(x, np.asarray(gate_w), np.asarray(e_bias))

    # dispatch: token lists per expert, sorted-stable by expert id
    flat_e = topk_idx.ravel()
    order = np.argsort(flat_e, kind="stable")
    pair_tok = order // TOP_K
    pair_w = (topk_w.ravel()[order] * ROUTED_SCALE).astype(np.float32)
    counts = np.bincount(flat_e, minlength=E)
    starts = np.zeros(E + 1, np.int64)
    np.cumsum(counts, out=starts[1:])

    # expert -> core assignment: pair largest with smallest
    by_count = np.argsort(-counts, kind="stable")
    slotA = by_count[:N_CORES]
    slotB = by_count[E - 1 : N_CORES - 1 : -1]  # reversed smallest half
    C1 = _cap(counts[slotA].max())
    C2 = _cap(counts[slotB].max())

    nc = _build_program(C1, C2)

    xhi, xlo = _hi_lo(x, SX)  # [T, D] e4m3
    xhiT = np.ascontiguousarray(xhi.T)  # [D, T]
    xloT = np.ascontiguousarray(xlo.T)

    ws1_p = _pack_w(np.asarray(ws_gate_up), SW)
    ws2_p = _pack_w(np.asarray(ws_down), SW)
    w_gate_up = np.asarray(w_gate_up)
    w_down = np.asarray(w_down)

    TS = T // N_CORES
    in_maps = []
    core_info = []
    for c in range(N_CORES):
        eA, eB = int(slotA[c]), int(slotB[c])
        m = {}
        info = []
        for slot, e_id, C, xt_name, wr_name in (
            (0, eA, C1, "xt1", "wr1"),
            (1, eB, C2, "xt2", "wr2"),
        ):
            idx = pair_tok[starts[e_id] : starts[e_id + 1]]
            w = pair_w[starts[e_id] : starts[e_id + 1]]
            n_e = len(idx)
            idx_pad = np.zeros(C, np.int64)
            idx_pad[:n_e] = idx
            w_pad = np.zeros(C, np.float32)
            w_pad[:n_e] = w * CY
            m[xt_name] = _pack_x(xhiT, xloT, idx_pad)
            m[wr_name] = np.ascontiguousarray(np.broadcast_to(w_pad, (P, C)))
            info.append((idx, n_e))
        m["xts"] = _pack_x(xhiT, xloT, np.arange(c * TS, (c + 1) * TS))
        m["w1a"] = _pack_w(w_gate_up[eA], SW)
        m["w2a"] = _pack_w(w_down[eA], SW)
        m["w1b"] = _pack_w(w_gate_up[eB], SW)
        m["w2b"] = _pack_w(w_down[eB], SW)
        m["ws1"] = ws1_p
        m["ws2"] = ws2_p
        in_maps.append(m)
        core_info.append(info)
    return nc, in_maps, core_info


def _combine(res_results, core_info):
    TS = T // N_CORES
    out = np.zeros((T, D), np.float32)
    for c in range(N_CORES):
        (idxA, nA), (idxB, nB) = core_info[c]
        out[idxA] += res_results[c]["y1"][:, :nA].astype(np.float32).T
        out[idxB] += res_results[c]["y2"][:, :nB].astype(np.float32).T
        out[c * TS : (c + 1) * TS] += res_results[c]["ys"].astype(np.float32).T
    return out


def kernel(hidden_states, gate_w, e_bias, w_gate_up, w_down, ws_gate_up, ws_down):
    nc, in_maps, core_info = _prepare(
        hidden_states, gate_w, e_bias, w_gate_up, w_down, ws_gate_up, ws_down
    )
    res = run_bass_kernel_spmd(nc, in_maps, list(range(N_CORES)))
    return _combine(res.results, core_info)


# revision 33
# speedup vs baseline: 1.3610x; 1.0010x over previous
"""DeepseekV2 MoE layer on 8 Trainium2 NeuronCores.

Strategy (expert-parallel, per the sharding hint):
  - Router gate + grouped top-k computed on host (0.03% of module FLOPs);
    it determines the dispatch, which IS the input sharding.
  - 16 routed experts paired big-count-with-small-count onto 8 cores
    (2 experts per core, token lists gathered host-side, padded to a
    shared per-slot capacity so all cores run one SPMD program).
  - Shared-expert MLP is data-parallel over tokens: each core runs
    T/8 = 512 tokens through the full shared MLP.
  - All matmuls run as fp8(e4m3) DoubleRow pairs with 3-term hi/lo error
    compensation: for every operand pair (W, X), W = W_hi + W_lo and
    X = X_hi + X_lo in scaled e4m3; the product is computed as
    W_hi.X_hi + (W_hi.X_lo + W_lo.X_hi), dropping the negligible
    W_lo.X_lo term. Each DoubleRow instruction carries two K=128
    products, so a K=256 contraction costs 3 instructions vs 2 for
    bf16 while retaining (slightly better than) bf16 accuracy.
  - f32 PSUM accumulation; bf16 outputs (combined in f32 on host).
"""

import sys

sys.path.insert(0, "/opt/trn_rl_repo")

import copy

import ml_dtypes
import numpy as np

import concourse.bass as bass
import concourse.mybir as mybir
import concourse.tile as tile
from concourse.bass_utils import run_bass_kernel_spmd

DT = mybir.dt
F8 = ml_dtypes.float8_e4m3
BF16 = ml_dtypes.bfloat16
DR = mybir.MatmulPerfMode.DoubleRow

T, D, E, I = 4096, 2048, 16, 1024
TOP_K, N_GROUP, TOPK_GROUP = 4, 4, 2
ROUTED_SCALE = 2.5
SHARED_I = 2048
N_CORES = 8
P = 128
NCH = 256  # token chunk (DoubleRow moving free = 2*NCH = 512 max)

SX = 16.0  # x scale into e4m3
SW = 512.0  # weight scale into e4m3
SH = 8.0  # h scale into e4m3
CU = SH / (SX * SW * SX * SW)  # ps_u -> u*SH/(SX*SW)
CY = 1.0 / (SH * SW)  # down psum descale


# ---------------------------------------------------------------- wait split
def _split_excess_waits(nc, limit=1):
    """This walrus build rejects >1 sync-wait command per instruction.
    Move excess waits onto fresh same-engine NOPs inserted just before."""
    template = bass.Bass(target_bir_lowering=False).sync.nop(nofuse=True).ins
    ctr = 0
    for bb in nc.main_func.blocks:
        out = []
        changed = False
        for ins in bb.instructions:
            si = ins.sync_info
            if si is not None and si.on_wait and len(si.on_wait) > limit:
                waits = list(si.on_wait)
                for w in waits[:-limit]:
                    ctr += 1
                    nop = copy.deepcopy(template)
                    nop.name = f"I-wsplit-{ctr}"
                    nop.engine = ins.engine
                    nop.bass_nofuse = True
                    nop.sync_info = mybir.SyncInfo(on_wait=[w], on_update=[])
                    nc.register_instruction(nop, overwrite=True)
                    out.append(nop)
                ins.sync_info = mybir.SyncInfo(
                    on_wait=waits[-limit:], on_update=list(si.on_update)
                )
                changed = True
            out.append(ins)
        if changed:
            bb.instructions = out
    return ctr


# ---------------------------------------------------------------- routing
def _gate_logits(x, gate_w):
    # Match the reference's jax-f32 CPU matmul as closely as possible.
    try:
        import jax
        import jax.numpy as jnp

        cpu = jax.devices("cpu")[0]
        with jax.default_device(cpu):
            return np.asarray(jnp.matmul(jnp.asarray(x), jnp.asarray(gate_w)))
    except Exception:
        return (x @ gate_w).astype(np.float32)


def _route(x, gate_w, e_bias):
    logits = _gate_logits(x, gate_w)  # [T, E] f32
    scores = (1.0 / (1.0 + np.exp(-logits))).astype(np.float32)
    sfc = scores + e_bias[None, :]
    grp = sfc.reshape(T, N_GROUP, E // N_GROUP)
    group_scores = np.sort(grp, axis=-1)[:, :, -2:].sum(-1)  # [T, G]
    group_idx = np.argsort(-group_scores, axis=-1, kind="stable")[:, :TOPK_GROUP]
    group_mask = np.zeros((T, N_GROUP), bool)
    group_mask[np.arange(T)[:, None], group_idx] = True
    expert_mask = np.repeat(group_mask, E // N_GROUP, axis=1)
    masked = np.where(expert_mask, sfc, -np.inf)
    topk_idx = np.argsort(-masked, axis=-1, kind="stable")[:, :TOP_K]  # [T, 4]
    topk_w = np.take_along_axis(scores, topk_idx, axis=1)
    topk_w = topk_w / topk_w.sum(axis=1, keepdims=True)
    return topk_idx.astype(np.int64), topk_w.astype(np.float32)


# ---------------------------------------------------------------- program
_PROGRAM_CACHE = {}


def _mm3(nc, ps, wt, xt, nk, tok, sz, first, last):
    """3-term compensated fp8 DoubleRow contraction over nk k-slices of 128.

    wt: stationary tile [P, nk, 2, P] with slot0=hi, slot1=lo.
    xt: moving tile [P, nk, 2, C] with slot0=lo, slot1=hi.
    ps: psum [P, NCH] (use [:, :sz]); tok = token offset into xt.
    """
    # hi*hi over k-slice pairs
    for j in range(nk // 2):
        nc.tensor.matmul(
            ps[:, :sz],
            wt[:, 2 * j : 2 * j + 2, 0, :],
            xt[:, 2 * j : 2 * j + 2, 1, tok : tok + sz],
            start=(first and j == 0),
            stop=False,
            perf_mode=DR,
        )
    # cross terms: (w_hi, w_lo) x (x_lo, x_hi) per k-slice
    for k in range(nk):
        nc.tensor.matmul(
            ps[:, :sz],
            wt[:, k, :, :],
            xt[:, k, :, tok : tok + sz],
            start=False,
            stop=(last and k == nk - 1),
            perf_mode=DR,
        )


def _emit_expert(nc, tc, pools, xt_h, w1_h, w2_h, wr_h, y_h, C, twoI, apply_wr,
                 bulk_q=False, first=False):
    n_d = D // P  # 16 contraction slices over D
    n_i = twoI // P  # gate_up output tiles
    n_h = n_i // 2  # h tiles (= I_/128)
    chunks = [(o, min(NCH, C - o)) for o in range(0, C, NCH)]

    (xt_pool, w1_pool, w2_pool, g_pool, h_pool, y_pool, wr_pool, sg_pool,
     tmp_pool, ps_gu, ps_dn) = pools

    # w1 slices consumed as (gate i, up i+n_h) pairs; prefetch 2 ahead so
    # slice loads are never just-in-time
    w1_order = []
    for ih in range(n_h):
        w1_order += [ih, ih + n_h]

    # All w1 loads go on the Pool queue.  Two effects: they never queue
    # behind the previous expert's w2 stream on SP, and — because the queue
    # is in-order and the w1 buffer rotation WAR-throttles it to compute
    # pace — the x bulk loads emitted after them are naturally delayed into
    # the mid-gate_up window, away from the congested phase boundaries.
    def load_w1(i):
        t = w1_pool.tile([P, n_d, 2, P], DT.float8e4, name="w1s")
        nc.gpsimd.dma_start(t[:], w1_h[i])
        return t

    n_pre = 6 if first else 3
    w1_tiles = {j: load_w1(w1_order[j]) for j in range(n_pre)}

    # whole-expert X tile [P, k-slice, (lo,hi), tok].  First expert: 4 chunky
    # loads (SP-issue rate is the cold-start limiter).  Later experts: per-d
    # slices on the Pool queue, so each transfer is short and never
    # head-of-line-blocks the latency-critical weight-slice stream on the
    # shared DMA engines.
    xt_q = nc.gpsimd if bulk_q else nc.sync
    xt_t = xt_pool.tile([P, n_d, 2, C], DT.float8e4, name="xt")
    bounds = [0, 2, 4, 8, 12, 16] if first else list(range(n_d + 1))
    for a, b in zip(bounds[:-1], bounds[1:]):
        xt_q.dma_start(xt_t[:, a:b, :, :], xt_h[:, a:b])

    wr_t = None
    if apply_wr:
        wr_t = wr_pool.tile([P, C], DT.float32, name="wr")
        xt_q.dma_start(wr_t[:], wr_h[:, :])

    h_t = h_pool.tile([P, n_h, 2, C], DT.float8e4, name="hil")

    # gate_up: process (gate i, up i+n_h) pairs so gate tiles die quickly
    for ih in range(n_h):
        gt_tiles = {}
        for half, i in ((0, ih), (1, ih + n_h)):
            idx = 2 * ih + half
            w1s = w1_tiles.pop(idx)
            if idx + n_pre < len(w1_order):
                w1_tiles[idx + n_pre] = load_w1(w1_order[idx + n_pre])
            for ci, (off, sz) in enumerate(chunks):
                ps = ps_gu.tile([P, NCH], DT.float32, name="psg")
                _mm3(nc, ps, w1s, xt_t, n_d, off, sz, True, True)
                if half == 0:
                    sg = sg_pool.tile([P, NCH], DT.float32, name="sg")
                    nc.scalar.activation(
                        sg[:, :sz], ps[:, :sz],
                        mybir.ActivationFunctionType.Sigmoid,
                        scale=1.0 / (SX * SW),
                    )
                    gt = g_pool.tile([P, NCH], DT.float32, name="gt")
                    nc.vector.tensor_mul(gt[:, :sz], ps[:, :sz], sg[:, :sz])
                    gt_tiles[ci] = gt
                else:
                    us = tmp_pool.tile([P, NCH], DT.float32, name="us")
                    nc.vector.tensor_scalar_mul(us[:, :sz], ps[:, :sz], CU)
                    th = tmp_pool.tile([P, NCH], DT.float32, name="th")
                    nc.vector.tensor_mul(th[:, :sz], gt_tiles[ci][:, :sz], us[:, :sz])
                    nc.scalar.copy(h_t[:, ih, 1, off : off + sz], th[:, :sz])
                    df = tmp_pool.tile([P, NCH], DT.float32, name="df")
                    nc.vector.tensor_sub(
                        df[:, :sz], th[:, :sz], h_t[:, ih, 1, off : off + sz]
                    )
                    nc.scalar.copy(h_t[:, ih, 0, off : off + sz], df[:, :sz])

    # down projection (w2 slices prefetched 2 ahead)
    def load_w2(d2):
        t = w2_pool.tile([P, n_h, 2, P], DT.float8e4, name="w2s")
        nc.sync.dma_start(t[:], w2_h[d2])
        return t

    w2_tiles = {0: load_w2(0), 1: load_w2(1)}
    for d2 in range(D // P):
        w2s = w2_tiles.pop(d2)
        if d2 + 2 < D // P:
            w2_tiles[d2 + 2] = load_w2(d2 + 2)
        ys = y_pool.tile([P, C], DT.bfloat16, name="ys")
        y_q = nc.scalar if d2 % 2 else nc.sync
        half = len(chunks) // 2
        for ci, (off, sz) in enumerate(chunks):
            ps = ps_dn.tile([P, NCH], DT.float32, name="psd")
            _mm3(nc, ps, w2s, h_t, n_h, off, sz, True, True)
            if apply_wr:
                nc.vector.tensor_mul(ys[:, off : off + sz], ps[:, :sz], wr_t[:, off : off + sz])
            else:
                nc.scalar.mul(ys[:, off : off + sz], ps[:, :sz], CY)
            if len(chunks) > 2 and ci == half - 1:
                mid = chunks[half][0]
                y_q.dma_start(y_h[d2 * P : (d2 + 1) * P, :mid], ys[:, :mid])
        lo = chunks[half][0] if len(chunks) > 2 else 0
        y_q.dma_start(y_h[d2 * P : (d2 + 1) * P, lo:], ys[:, lo:])


def _build_program(C1, C2):
    key = (C1, C2)
    if key in _PROGRAM_CACHE:
        return _PROGRAM_CACHE[key]

    nc = bass.Bass(target_bir_lowering=False)
    TS = T // N_CORES  # shared tokens per core
    n_d = D // P

    xt1 = nc.dram_tensor("xt1", [P, n_d, 2, C1], DT.float8e4, kind="ExternalInput")
    xt2 = nc.dram_tensor("xt2", [P, n_d, 2, C2], DT.float8e4, kind="ExternalInput")
    xts = nc.dram_tensor("xts", [P, n_d, 2, TS], DT.float8e4, kind="ExternalInput")
    w1a = nc.dram_tensor("w1a", [2 * I // P, P, n_d, 2, P], DT.float8e4, kind="ExternalInput")
    w2a = nc.dram_tensor("w2a", [D // P, P, I // P, 2, P], DT.float8e4, kind="ExternalInput")
    w1b = nc.dram_tensor("w1b", [2 * I // P, P, n_d, 2, P], DT.float8e4, kind="ExternalInput")
    w2b = nc.dram_tensor("w2b", [D // P, P, I // P, 2, P], DT.float8e4, kind="ExternalInput")
    ws1 = nc.dram_tensor("ws1", [2 * SHARED_I // P, P, n_d, 2, P], DT.float8e4, kind="ExternalInput")
    ws2 = nc.dram_tensor("ws2", [D // P, P, SHARED_I // P, 2, P], DT.float8e4, kind="ExternalInput")
    wr1 = nc.dram_tensor("wr1", [P, C1], DT.float32, kind="ExternalInput")
    wr2 = nc.dram_tensor("wr2", [P, C2], DT.float32, kind="ExternalInput")
    y1 = nc.dram_tensor("y1", [D, C1], DT.bfloat16, kind="ExternalOutput")
    y2 = nc.dram_tensor("y2", [D, C2], DT.bfloat16, kind="ExternalOutput")
    ys = nc.dram_tensor("ys", [D, TS], DT.bfloat16, kind="ExternalOutput")

    with tile.TileContext(nc) as tc:
        with (
            tc.tile_pool(name="xt", bufs=2) as xt_pool,
            tc.tile_pool(name="w1p", bufs=7) as w1_pool,
            tc.tile_pool(name="w2p", bufs=4) as w2_pool,
            tc.tile_pool(name="gp", bufs=8) as g_pool,
            tc.tile_pool(name="hp", bufs=2) as h_pool,
            tc.tile_pool(name="yp", bufs=3) as y_pool,
            tc.tile_pool(name="wrp", bufs=2) as wr_pool,
            tc.tile_pool(name="sgp", bufs=3) as sg_pool,
            tc.tile_pool(name="tmp", bufs=4) as tmp_pool,
            tc.tile_pool(name="psgu", bufs=4, space="PSUM") as ps_gu,
            tc.tile_pool(name="psdn", bufs=4, space="PSUM") as ps_dn,
        ):
            pools = (xt_pool, w1_pool, w2_pool, g_pool, h_pool, y_pool, wr_pool,
                     sg_pool, tmp_pool, ps_gu, ps_dn)
            # shared first: its small x-load makes the cold-start short, and
            # the routed experts' larger input streams prefetch underneath it
            _emit_expert(nc, tc, pools, xts, ws1, ws2, None, ys, TS, 2 * SHARED_I, False,
                         first=True)
            _emit_expert(nc, tc, pools, xt1, w1a, w2a, wr1, y1, C1, 2 * I, True,
                         bulk_q=True)
            _emit_expert(nc, tc, pools, xt2, w1b, w2b, wr2, y2, C2, 2 * I, True,
                         bulk_q=True)

    _split_excess_waits(nc, limit=1)
    _PROGRAM_CACHE[key] = nc
    return nc


# ---------------------------------------------------------------- packing
def _hi_lo(a, scale):
    s = (a * scale).astype(np.float32)
    hi = s.astype(F8)
    lo = (s - hi.astype(np.float32)).astype(F8)
    return hi, lo


def _pack_w(w, scale):
    """w [K, F] f32 -> [F/P, P(k-in-slice), K/P, 2(hi,lo), P(feat)] e4m3."""
    K, F = w.shape
    n_k, n_f = K // P, F // P
    hi, lo = _hi_lo(w, scale)

    def arr(a):
        return a.reshape(n_k, P, n_f, P).transpose(2, 1, 0, 3)

    out = np.empty((n_f, P, n_k, 2, P), F8)
    out[:, :, :, 0, :] = arr(hi)
    out[:, :, :, 1, :] = arr(lo)
    return np.ascontiguousarray(out)


def _pack_x(xhiT, xloT, cols):
    """xhiT/xloT [D, T] e4m3 + column index -> [P, D/P, 2(lo,hi), C]."""
    n_d = D // P
    C = len(cols)
    out = np.empty((P, n_d, 2, C), F8)
    out[:, :, 0, :] = xloT[:, cols].reshape(n_d, P, C).transpose(1, 0, 2)
    out[:, :, 1, :] = xhiT[:, cols].reshape(n_d, P, C).transpose(1, 0, 2)
    return np.ascontiguousarray(out)


def _cap(n):
    # exact capacity; keep a small floor so degenerate routings stay sane
    return max(P, int(n))


# ---------------------------------------------------------------- kernel
def _prepare(hidden_states, gate_w, e_bias, w_gate_up, w_down, ws_gate_up, ws_down):
    x = np.asarray(hidden_states, dtype=np.float32)
    topk_idx, topk_w = _route(x, np.asarray(gate_w), np.asarray(e_bias))

    # dispatch: token lists per expert, sorted-stable by expert id
    flat_e = topk_idx.ravel()
    order = np.argsort(flat_e, kind="stable")
    pair_tok = order // TOP_K
    pair_w = (topk_w.ravel()[order] * ROUTED_SCALE).astype(np.float32)
    counts = np.bincount(flat_e, minlength=E)
    starts = np.zeros(E + 1, np.int64)
    np.cumsum(counts, out=starts[1:])

    # expert -> core assignment: pair largest with smallest
    by_count = np.argsort(-counts, kind="stable")
    slotA = by_count[:N_CORES]
    slotB = by_count[E - 1 : N_CORES - 1 : -1]  # reversed smallest half
    C1 = _cap(counts[slotA].max())
    C2 = _cap(counts[slotB].max())

    nc = _build_program(C1, C2)

    xhi, xlo = _hi_lo(x, SX)  # [T, D] e4m3
    xhiT = np.ascontiguousarray(xhi.T)  # [D, T]
    xloT = np.ascontiguousarray(xlo.T)

    ws1_p = _pack_w(np.asarray(ws_gate_up), SW)
    ws2_p = _pack_w(np.asarray(ws_down), SW)
    w_gate_up = np.asarray(w_gate_up)
    w_down = np.asarray(w_down)

    TS = T // N_CORES
    in_maps = []
    core_info = []
    for c in range(N_CORES):
        eA, eB = int(slotA[c]), int(slotB[c])
        m = {}
        info = []
        for slot, e_id, C, xt_name, wr_name in (
            (0, eA, C1, "xt1", "wr1"),
            (1, eB, C2, "xt2", "wr2"),
        ):
            idx = pair_tok[starts[e_id] : starts[e_id + 1]]
            w = pair_w[starts[e_id] : starts[e_id + 1]]
            n_e = len(idx)
            idx_pad = np.zeros(C, np.int64)
            idx_pad[:n_e] = idx
            w_pad = np.zeros(C, np.float32)
            w_pad[:n_e] = w * CY
            m[xt_name] = _pack_x(xhiT, xloT, idx_pad)
            m[wr_name] = np.ascontiguousarray(np.broadcast_to(w_pad, (P, C)))
            info.append((idx, n_e))
        m["xts"] = _pack_x(xhiT, xloT, np.arange(c * TS, (c + 1) * TS))
        m["w1a"] = _pack_w(w_gate_up[eA], SW)
        m["w2a"] = _pack_w(w_down[eA], SW)
        m["w1b"] = _pack_w(w_gate_up[eB], SW)
        m["w2b"] = _pack_w(w_down[eB], SW)
        m["ws1"] = ws1_p
        m["ws2"] = ws2_p
        in_maps.append(m)
        core_info.append(info)
    return nc, in_maps, core_info


def _combine(res_results, core_info):
    TS = T // N_CORES
    out = np.zeros((T, D), np.float32)
    for c in range(N_CORES):
        (idxA, nA), (idxB, nB) = core_info[c]
        out[idxA] += res_results[c]["y1"][:, :nA].astype(np.float32).T
        out[idxB] += res_results[c]["y2"][:, :nB].astype(np.float32).T
        out[c * TS : (c + 1) * TS] += res_results[c]["ys"].astype(np.float32).T
    return out


def kernel(hidden_states, gate_w, e_bias, w_gate_up, w_down, ws_gate_up, ws_down):
    nc, in_maps, core_info = _prepare(
        hidden_states, gate_w, e_bias, w_gate_up, w_down, ws_gate_up, ws_down
    )
    res = run_bass_kernel_spmd(nc, in_maps, list(range(N_CORES)))
    return _combine(res.results, core_info)
